# revision 35
# baseline (speedup 1.0000x reference)
"""DLoRF low-rank linear kernel for Trainium2 (8 NeuronCores, SPMD).

Computes  out = x @ U @ diag(s * mask) @ V.T  for
  x [8, 2048, 4096] f32, U [4096, 512], V [4096, 512], s/mask [512].

Strategy: data-parallel over the batch dim (one batch element per core).
Host folds diag(s*mask) into U (U_s = U * s_masked) and pre-transposes
V (Vt = V.T), both tiny. Per core:

  phase 1: stream x in natural layout, transpose 128x128 tiles on the
           PE (identity matmul) to get x.T tiles (feature-major), then
           GEMM1: tT[k', tok] += U_s[feat, k'].T @ xT[feat, tok]
  phase 2: GEMM2: out[tok, O] += tT[k', tok].T @ Vt[k', O], streamed
           over O chunks, DMA out.

Matmuls run as float32r (TF32-like: fp32 bits, mantissa rounded to
~12 bits inside the PE) which streams at 1 cycle/row -- 4x faster than
exact fp32. Measured rel-l2 error per GEMM ~1.5e-4.
"""

import numpy as np

import concourse.bacc as bacc
import concourse.mybir as mybir
import concourse.tile as tile
from concourse.bass import _add_dep_helper
from concourse.bass_utils import run_bass_kernel_spmd

B, S, IN_F, OUT_F, KR = 8, 2048, 4096, 4096, 512
P = 128
N_CORES = 8
KT = IN_F // P  # 32 feature tiles (contraction of GEMM1)
MT = KR // P  # 4 rank tiles (contraction of GEMM2)
CW = 256  # token chunk width (moving free dim of GEMM1)
CH = S // CW  # 8 chunks
OW = 512  # out-feature chunk width (moving free dim of GEMM2)
OC = OUT_F // OW  # 8 chunks

F32 = mybir.dt.float32
F32R = mybir.dt.float32r


def build(dt_mm=F32R, f32r_transpose=True):
    nc = bacc.Bacc()
    # dtype of the transpose path (x natural tiles, transpose psum)
    dt_tr = dt_mm if f32r_transpose else F32
    x_d = nc.declare_dram_parameter("x", [S, IN_F], dt_tr, isOutput=False)
    # weights arrive host-pre-arranged in SBUF layout (partition-major)
    # so the resident-weight DMAs are contiguous per partition
    us_d = nc.declare_dram_parameter("us", [P, KT, KR], dt_mm, isOutput=False)
    vt_d = nc.declare_dram_parameter("vt", [P, MT, OUT_F], dt_mm, isOutput=False)
    id_d = nc.declare_dram_parameter("ident", [P, P], dt_tr, isOutput=False)
    out_d = nc.declare_dram_parameter("out", [S, OUT_F], F32, isOutput=True)

    with tile.TileContext(nc) as tc:
        with (
            tc.tile_pool(name="const", bufs=1) as constp,
            tc.tile_pool(name="wpool", bufs=1) as wpool,
            tc.tile_pool(name="xnat", bufs=6) as xnat_p,
            tc.tile_pool(name="xt", bufs=1) as xt_p,
            tc.tile_pool(name="tt", bufs=3) as tt_p,
            tc.tile_pool(name="tnat", bufs=2) as tnat_p,
            tc.tile_pool(name="ostage", bufs=3) as ostage_p,
            tc.tile_pool(name="tps", bufs=2, space="PSUM") as tps,
            tc.tile_pool(name="ps1", bufs=2, space="PSUM") as ps1,
            tc.tile_pool(name="ps2", bufs=3, space="PSUM") as ps2,
        ):
            # identity for PE transposes, loaded from DRAM on the sync
            # ring ahead of the first x tile (lands in ~1us)
            ident_mm = constp.tile([P, P], dt_tr)
            nc.sync.dma_start(ident_mm[:], id_d[:])

            # Weights stay resident all kernel, on the gpsimd (SWDGE)
            # queue -- the sync HWDGE ring is reserved for x streaming
            # and the scalar HWDGE ring for output stores. The 16MB of
            # weights would starve the latency-critical early x loads
            # (HBM is ~358GB/s per core), so V.T pieces are explicitly
            # sequenced behind chunk 1's x loads via dep edges; GEMM2
            # is skewed two chunks behind transpose/GEMM1 so V.T has
            # ~60us to arrive.
            us_t = wpool.tile([P, KT, KR], dt_mm)
            vt_full = wpool.tile([P, MT, OUT_F], dt_mm)
            us_dmas = []
            for h in range(8):
                us_dmas.append(
                    nc.gpsimd.dma_start(
                        us_t[:, h * 4 : (h + 1) * 4],
                        us_d[:, h * 4 : (h + 1) * 4],
                    )
                )
            vt_dmas = [
                nc.gpsimd.dma_start(
                    vt_full[:, :, oc * OW : (oc + 1) * OW],
                    vt_d[:, :, oc * OW : (oc + 1) * OW],
                )
                for oc in range(OC)
            ]

            # Pipeline per 256-token chunk, with GEMM2 skewed one chunk
            # behind transpose+GEMM1 so the PE has transpose/GEMM1 work
            # (needing only x and U_s) while the 8MB of V.T still
            # streams in during the first ~45us.
            xn_dmas = {}

            def transpose_and_gemm1(c):
                xt_tile = xt_p.tile([P, KT, CW], dt_mm, tag="xt")
                for ts in range(CW // P):
                    tok0 = c * CW + ts * P
                    for fq in range(4):  # 1024-feature quarters
                        xn = xnat_p.tile([P, IN_F // 4], dt_tr, tag="xn")
                        xn_dmas[(c, ts, fq)] = nc.sync.dma_start(
                            xn[:],
                            x_d[tok0 : tok0 + P, fq * 1024 : (fq + 1) * 1024],
                        )
                        for q in range(2):
                            tp = tps.tile([P, 512], dt_tr, tag="tp")
                            for j in range(4):
                                nc.tensor.transpose(
                                    tp[:, j * P : (j + 1) * P],
                                    xn[:, (q * 4 + j) * P : (q * 4 + j + 1) * P],
                                    ident_mm,
                                )
                            kt0 = fq * 8 + q * 4
                            nc.vector.tensor_copy(
                                xt_tile[:, kt0 : kt0 + 4, ts * P : (ts + 1) * P],
                                tp.rearrange("p (j c) -> p j c", j=4),
                            )
                # GEMM1 with U_s as the 512-wide moving operand: psum
                # gets t natural [tok, k'], which is then PE-transposed
                # (cheap: 4 tiles per token block) into tt for GEMM2.
                tt_c = tt_p.tile([P, MT, CW], dt_mm, tag="tt")
                for ts in range(CW // P):
                    p1 = ps1.tile([P, KR], F32, tag="p1")
                    for kt in range(KT):
                        nc.tensor.matmul(
                            p1[:],
                            xt_tile[:, kt, ts * P : (ts + 1) * P],
                            us_t[:, kt, :],
                            start=(kt == 0),
                            stop=(kt == KT - 1),
                        )
                    tn = tnat_p.tile([P, KR], dt_mm, tag="tn")
                    nc.scalar.copy(tn[:], p1[:])
                    tp2 = tps.tile([P, 512], dt_tr, tag="tp")
                    for m in range(MT):
                        nc.tensor.transpose(
                            tp2[:, m * P : (m + 1) * P],
                            tn[:, m * P : (m + 1) * P],
                            ident_mm,
                        )
                    nc.vector.tensor_copy(
                        tt_c[:, :, ts * P : (ts + 1) * P],
                        tp2.rearrange("p (m c) -> p m c", m=4),
                    )
                return tt_c

            def gemm2(c, tt_c):
                for ts in range(CW // P):
                    tok0 = c * CW + ts * P
                    for oc in range(OC):
                        p2 = ps2.tile([P, OW], F32, tag="p2")
                        for m in range(MT):
                            nc.tensor.matmul(
                                p2[:],
                                tt_c[:, m, ts * P : (ts + 1) * P],
                                vt_full[:, m, oc * OW : (oc + 1) * OW],
                                start=(m == 0),
                                stop=(m == MT - 1),
                            )
                        ost = ostage_p.tile([P, OW], F32, tag="ost")
                        nc.scalar.copy(ost[:], p2[:])
                        nc.scalar.dma_start(
                            out_d[tok0 : tok0 + P, oc * OW : (oc + 1) * OW],
                            ost[:],
                        )

            SKEW = 2
            tts = {}
            for c in range(CH + SKEW):
                if c < CH:
                    tts[c] = transpose_and_gemm1(c)
                if c == 0:
                    # The per-core HBM stream is effectively serial, so
                    # sequence weight loads behind the x tiles that the
                    # PE needs first: only us piece m0 races chunk 0's x.
                    for udma in us_dmas[2:]:
                        _add_dep_helper(
                            udma.ins,
                            xn_dmas[(0, 0, 1)].ins,
                            sync=True,
                            reason="stagger us loads behind first x tiles",
                        )
                if c == 1:
                    # V.T streams in only after chunk 1's x requests, so
                    # the early HBM window goes to x + U_s
                    for vdma in vt_dmas:
                        _add_dep_helper(
                            vdma.ins,
                            xn_dmas[(1, 1, 3)].ins,
                            sync=True,
                            reason="stagger vt loads behind early x stream",
                        )
                if c >= SKEW:
                    gemm2(c - SKEW, tts.pop(c - SKEW))
    nc.finalize()
    return nc


_NC_CACHE = {}


def _get_nc():
    key = "main"
    if key not in _NC_CACHE:
        _NC_CACHE[key] = build()
    return _NC_CACHE[key]


def kernel(x, U, V, s, mask, _trace=False, _trace_kwargs=None):
    s_masked = (s.astype(np.float32) * mask.astype(np.float32)).astype(np.float32)
    U_s = U.astype(np.float32) * s_masked[None, :]
    Vt = V.astype(np.float32).T
    # pre-arrange weights into the kernel's partition-major SBUF layout
    us_prep = np.ascontiguousarray(
        U_s.reshape(KT, P, KR).transpose(1, 0, 2)
    )  # [P, KT, KR]
    vt_prep = np.ascontiguousarray(
        Vt.reshape(MT, P, OUT_F).transpose(1, 0, 2)
    )  # [P, MT, OUT_F]
    ident = np.eye(P, dtype=np.float32)
    nc = _get_nc()
    in_maps = [
        {
            "x": np.ascontiguousarray(x[b]),
            "us": us_prep,
            "vt": vt_prep,
            "ident": ident,
        }
        for b in range(B)
    ]
    res = run_bass_kernel_spmd(
        nc, in_maps, list(range(N_CORES)), trace=_trace, **(_trace_kwargs or {})
    )
    out = np.stack([res.results[b]["out"] for b in range(B)], axis=0)
    if _trace:
        return out, res
    return out


# revision 37
# speedup vs baseline: 1.0332x; 1.0332x over previous
"""DLoRF low-rank linear kernel for Trainium2 (8 NeuronCores, SPMD).

Computes  out = x @ U @ diag(s * mask) @ V.T  for
  x [8, 2048, 4096] f32, U [4096, 512], V [4096, 512], s/mask [512].

Strategy: data-parallel over the batch dim (one batch element per core).
Host folds diag(s*mask) into U (U_s = U * s_masked) and pre-transposes
V (Vt = V.T), both tiny. Per core:

  phase 1: stream x in natural layout, transpose 128x128 tiles on the
           PE (identity matmul) to get x.T tiles (feature-major), then
           GEMM1: tT[k', tok] += U_s[feat, k'].T @ xT[feat, tok]
  phase 2: GEMM2: out[tok, O] += tT[k', tok].T @ Vt[k', O], streamed
           over O chunks, DMA out.

Matmuls run as float32r (TF32-like: fp32 bits, mantissa rounded to
~12 bits inside the PE) which streams at 1 cycle/row -- 4x faster than
exact fp32. Measured rel-l2 error per GEMM ~1.5e-4.
"""

import numpy as np

import concourse.bacc as bacc
import concourse.mybir as mybir
import concourse.tile as tile
from concourse.bass import _add_dep_helper
from concourse.bass_utils import run_bass_kernel_spmd

B, S, IN_F, OUT_F, KR = 8, 2048, 4096, 4096, 512
P = 128
N_CORES = 8
KT = IN_F // P  # 32 feature tiles (contraction of GEMM1)
MT = KR // P  # 4 rank tiles (contraction of GEMM2)
CW = 256  # token chunk width (moving free dim of GEMM1)
CH = S // CW  # 8 chunks
OW = 512  # out-feature chunk width (moving free dim of GEMM2)
OC = OUT_F // OW  # 8 chunks

F32 = mybir.dt.float32
F32R = mybir.dt.float32r


def build(dt_mm=F32R, f32r_transpose=True):
    nc = bacc.Bacc()
    # dtype of the transpose path (x natural tiles, transpose psum)
    dt_tr = dt_mm if f32r_transpose else F32
    x_d = nc.declare_dram_parameter("x", [S, IN_F], dt_tr, isOutput=False)
    # weights arrive host-pre-arranged in SBUF layout (partition-major)
    # so the resident-weight DMAs are contiguous per partition
    us_d = nc.declare_dram_parameter("us", [P, MT, KT, P], dt_mm, isOutput=False)
    vt_d = nc.declare_dram_parameter("vt", [P, MT, OUT_F], dt_mm, isOutput=False)
    id_d = nc.declare_dram_parameter("ident", [P, P], dt_tr, isOutput=False)
    out_d = nc.declare_dram_parameter("out", [S, OUT_F], F32, isOutput=True)

    with tile.TileContext(nc) as tc:
        with (
            tc.tile_pool(name="const", bufs=1) as constp,
            tc.tile_pool(name="wpool", bufs=1) as wpool,
            tc.tile_pool(name="xnat", bufs=6) as xnat_p,
            tc.tile_pool(name="xt", bufs=1) as xt_p,
            tc.tile_pool(name="tt", bufs=3) as tt_p,
            tc.tile_pool(name="ostage", bufs=4) as ostage_p,
            tc.tile_pool(name="tps", bufs=2, space="PSUM") as tps,
            tc.tile_pool(name="ps1", bufs=3, space="PSUM") as ps1,
            tc.tile_pool(name="ps2", bufs=3, space="PSUM") as ps2,
        ):
            # identity for PE transposes, loaded from DRAM on the sync
            # ring ahead of the first x tile (lands in ~1us)
            ident_mm = constp.tile([P, P], dt_tr)
            nc.sync.dma_start(ident_mm[:], id_d[:])

            # Weights stay resident all kernel, on the gpsimd (SWDGE)
            # queue -- the sync HWDGE ring is reserved for x streaming
            # and the scalar HWDGE ring for output stores. The 16MB of
            # weights would starve the latency-critical early x loads
            # (HBM is ~358GB/s per core), so V.T pieces are explicitly
            # sequenced behind chunk 1's x loads via dep edges; GEMM2
            # is skewed two chunks behind transpose/GEMM1 so V.T has
            # ~60us to arrive.
            us_t = wpool.tile([P, MT, KT, P], dt_mm)
            vt_full = wpool.tile([P, MT, OUT_F], dt_mm)
            us_dmas = []
            for m in range(MT):
                for h in range(2):
                    us_dmas.append(
                        nc.gpsimd.dma_start(
                            us_t[:, m, h * 16 : (h + 1) * 16],
                            us_d[:, m, h * 16 : (h + 1) * 16],
                        )
                    )
            vt_dmas = [
                nc.gpsimd.dma_start(
                    vt_full[:, :, oc * OW : (oc + 1) * OW],
                    vt_d[:, :, oc * OW : (oc + 1) * OW],
                )
                for oc in range(OC)
            ]

            # Pipeline per 256-token chunk, with GEMM2 skewed one chunk
            # behind transpose+GEMM1 so the PE has transpose/GEMM1 work
            # (needing only x and U_s) while the 8MB of V.T still
            # streams in during the first ~45us.
            xn_dmas = {}

            def transpose_and_gemm1(c):
                xt_tile = xt_p.tile([P, KT, CW], dt_mm, tag="xt")
                for ts in range(CW // P):
                    tok0 = c * CW + ts * P
                    for fq in range(4):  # 1024-feature quarters
                        xn = xnat_p.tile([P, IN_F // 4], dt_tr, tag="xn")
                        xn_dmas[(c, ts, fq)] = nc.sync.dma_start(
                            xn[:],
                            x_d[tok0 : tok0 + P, fq * 1024 : (fq + 1) * 1024],
                        )
                        for q in range(2):
                            tp = tps.tile([P, 512], dt_tr, tag="tp")
                            for j in range(4):
                                nc.tensor.transpose(
                                    tp[:, j * P : (j + 1) * P],
                                    xn[:, (q * 4 + j) * P : (q * 4 + j + 1) * P],
                                    ident_mm,
                                )
                            kt0 = fq * 8 + q * 4
                            nc.vector.tensor_copy(
                                xt_tile[:, kt0 : kt0 + 4, ts * P : (ts + 1) * P],
                                tp.rearrange("p (j c) -> p j c", j=4),
                            )
                tt_c = tt_p.tile([P, MT, CW], dt_mm, tag="tt")
                for m in range(MT):
                    p1 = ps1.tile([P, CW], F32, tag="p1")
                    for kt in range(KT):
                        nc.tensor.matmul(
                            p1[:],
                            us_t[:, m, kt, :],
                            xt_tile[:, kt, :],
                            start=(kt == 0),
                            stop=(kt == KT - 1),
                        )
                    nc.scalar.copy(tt_c[:, m, :], p1[:])
                return tt_c

            def gemm2(c, tt_c):
                for ts in range(CW // P):
                    tok0 = c * CW + ts * P
                    for oc in range(OC):
                        p2 = ps2.tile([P, OW], F32, tag="p2")
                        for m in range(MT):
                            nc.tensor.matmul(
                                p2[:],
                                tt_c[:, m, ts * P : (ts + 1) * P],
                                vt_full[:, m, oc * OW : (oc + 1) * OW],
                                start=(m == 0),
                                stop=(m == MT - 1),
                            )
                        ost = ostage_p.tile([P, OW], F32, tag="ost")
                        # split psum evicts across ACT and DVE so neither
                        # engine gates the PE's psum-buffer recycling
                        if oc % 2 == 0:
                            nc.scalar.copy(ost[:], p2[:])
                        else:
                            nc.vector.tensor_copy(ost[:], p2[:])
                        nc.scalar.dma_start(
                            out_d[tok0 : tok0 + P, oc * OW : (oc + 1) * OW],
                            ost[:],
                        )

            SKEW = 2
            tts = {}
            for c in range(CH + SKEW):
                if c < CH:
                    tts[c] = transpose_and_gemm1(c)
                if c == 0:
                    # The per-core HBM stream is effectively serial, so
                    # sequence weight loads behind the x tiles that the
                    # PE needs first: only us piece m0 races chunk 0's x.
                    for udma in us_dmas[2:]:
                        _add_dep_helper(
                            udma.ins,
                            xn_dmas[(0, 0, 1)].ins,
                            sync=True,
                            reason="stagger us loads behind first x tiles",
                        )
                if c == 1:
                    # V.T streams in only after chunk 1's x requests, so
                    # the early HBM window goes to x + U_s
                    for vdma in vt_dmas:
                        _add_dep_helper(
                            vdma.ins,
                            xn_dmas[(1, 1, 3)].ins,
                            sync=True,
                            reason="stagger vt loads behind early x stream",
                        )
                if c >= SKEW:
                    gemm2(c - SKEW, tts.pop(c - SKEW))
    nc.finalize()
    return nc


_NC_CACHE = {}


def _get_nc():
    key = "main"
    if key not in _NC_CACHE:
        _NC_CACHE[key] = build()
    return _NC_CACHE[key]


def kernel(x, U, V, s, mask, _trace=False, _trace_kwargs=None):
    s_masked = (s.astype(np.float32) * mask.astype(np.float32)).astype(np.float32)
    U_s = U.astype(np.float32) * s_masked[None, :]
    Vt = V.astype(np.float32).T
    # pre-arrange weights into the kernel's partition-major SBUF layout
    us_prep = np.ascontiguousarray(
        U_s.reshape(KT, P, MT, P).transpose(1, 2, 0, 3)
    )  # [P, MT, KT, P]
    vt_prep = np.ascontiguousarray(
        Vt.reshape(MT, P, OUT_F).transpose(1, 0, 2)
    )  # [P, MT, OUT_F]
    ident = np.eye(P, dtype=np.float32)
    nc = _get_nc()
    in_maps = [
        {
            "x": np.ascontiguousarray(x[b]),
            "us": us_prep,
            "vt": vt_prep,
            "ident": ident,
        }
        for b in range(B)
    ]
    res = run_bass_kernel_spmd(
        nc, in_maps, list(range(N_CORES)), trace=_trace, **(_trace_kwargs or {})
    )
    out = np.stack([res.results[b]["out"] for b in range(B)], axis=0)
    if _trace:
        return out, res
    return out
